# revision 29
# baseline (speedup 1.0000x reference)
"""Multi-head attention (B=1, S=4096, D=512, H=8, causal) on 8 trn2 NeuronCores.

Sharding: one head per core (tensor parallel). Each core:
  - computes its head's q/k/v projections from host-transposed inputs
    (qT/kT in [d, S] layout, v in [S, d] layout -- no on-chip transposes),
  - computes scores = softmax(q k^T / d) in [q, k] layout for the scores
    output (exp via ScalarE with accum_out rowsums; causal blocks skipped,
    unwritten output regions are guaranteed-zero),
  - computes the same scores in [k, q] layout to feed P^T directly into the
    attention-value matmul, then its slice of the Wo projection.
Host gathers: scores stacked over heads; partial outputs summed.

All matmuls run in fp32r (fp32 storage, 11-bit mantissa products, fp32
accumulation) at full PE rate; inputs are pre-rounded on host.
"""
import functools
import numpy as np
from contextlib import ExitStack

import concourse.bass as bass
import concourse.tile as tile
import concourse.mybir as mybir
from concourse import bacc
from concourse.bass_utils import run_bass_kernel_spmd

S = 4096
DM = 512
H = 8
DH = DM // H          # 64
NCHUNK = S // 128     # 32 q-chunks of 128 rows
NGRP = S // 512       # 8 groups of 512
NMT = DM // 128       # 4 m-tiles in the model dim

f32 = mybir.dt.float32
f32r = mybir.dt.float32r

TRACE = False          # set by test harness to capture an NTFF profile
LAST_RESULTS = None    # BassKernelResults of the last device run
_last_in_maps = None


def _round_fp32r(x: np.ndarray) -> np.ndarray:
    """Round fp32 -> fp32r (sign + 8e + 11m in the top 20 bits, RNE)."""
    u = np.ascontiguousarray(x, dtype=np.float32).view(np.uint32)
    bias = ((u >> 12) & 1) + np.uint32(0x7FF)
    return (((u + bias) >> 12) << 12).view(np.float32)


ORDER = dict(b2_split=True)

BUILD_CFG = dict(
    xt_bufs=6, row_bufs=4, est_bufs=4, ysb_bufs=2, otsb_bufs=2, sm_bufs=4,
    psa_bufs=3, psb_bufs=2, dve_copies=True,
)


@functools.lru_cache(maxsize=1)
def _build():
    return _build_cfg(**BUILD_CFG)


def _build_cfg(xt_bufs, row_bufs, est_bufs, ysb_bufs, otsb_bufs, sm_bufs,
               psa_bufs, psb_bufs, dve_copies):
    nc = bacc.Bacc("TRN2", target_bir_lowering=False, debug=False, num_devices=8)

    xtq_d = nc.dram_tensor("xtq", [DM, S], f32r, kind="ExternalInput")
    xtk_d = nc.dram_tensor("xtk", [DM, S], f32r, kind="ExternalInput")
    xtv_d = nc.dram_tensor("xtv", [DM, S], f32r, kind="ExternalInput")
    # weight slices pre-arranged on host to [128, NMT*DH]: col m*DH+d = W[m*128+p, d]
    wq_d = nc.dram_tensor("wq", [128, NMT * DH], f32r, kind="ExternalInput")
    wk_d = nc.dram_tensor("wk", [128, NMT * DH], f32r, kind="ExternalInput")
    wv_d = nc.dram_tensor("wv", [128, NMT * DH], f32r, kind="ExternalInput")
    wo_d = nc.dram_tensor("wo", [DH, DM], f32r, kind="ExternalInput")
    cm_d = nc.dram_tensor("cmask", [128, 128], f32, kind="ExternalInput")
    cmt_d = nc.dram_tensor("cmaskt", [128, 128], f32, kind="ExternalInput")

    scores_d = nc.dram_tensor("scores", [S, S], f32, kind="ExternalOutput")
    y_d = nc.dram_tensor("y", [S, DM], f32, kind="ExternalOutput")

    Exp = mybir.ActivationFunctionType.Exp
    AX = mybir.AxisListType.X

    with tile.TileContext(nc) as tc, ExitStack() as ctx:
        sb = ctx.enter_context(tc.tile_pool(name="sb", bufs=1))
        xt_pool = ctx.enter_context(tc.tile_pool(name="xt", bufs=xt_bufs))
        row_pool = ctx.enter_context(tc.tile_pool(name="row", bufs=row_bufs))
        est_pool = ctx.enter_context(tc.tile_pool(name="est", bufs=est_bufs))
        ysb_pool = ctx.enter_context(tc.tile_pool(name="ysb", bufs=ysb_bufs))
        ot_sb_pool = ctx.enter_context(tc.tile_pool(name="otsb", bufs=otsb_bufs))
        sm_pool = ctx.enter_context(tc.tile_pool(name="sm", bufs=sm_bufs))
        ps_a = ctx.enter_context(tc.tile_pool(name="psa", bufs=psa_bufs, space="PSUM"))
        ps_b = ctx.enter_context(tc.tile_pool(name="psb", bufs=psb_bufs, space="PSUM"))
        _copy = nc.vector.tensor_copy if dve_copies else (
            lambda out, in_: nc.scalar.copy(out, in_))

        wq_t = sb.tile([128, NMT * DH], f32r, tag="wq")
        wk_t = sb.tile([128, NMT * DH], f32r, tag="wk")
        wv_t = sb.tile([128, NMT * DH], f32r, tag="wv")
        wo_t = sb.tile([DH, DM], f32r, tag="wo")
        cm_t = sb.tile([128, 128], f32, tag="cm")
        cmt_t = sb.tile([128, 128], f32, tag="cmt")
        nc.sync.dma_start(wq_t[:], wq_d.ap())
        nc.sync.dma_start(wk_t[:], wk_d.ap())
        nc.sync.dma_start(wv_t[:], wv_d.ap())
        nc.sync.dma_start(wo_t[:], wo_d.ap())
        nc.sync.dma_start(cm_t[:], cm_d.ap())
        nc.sync.dma_start(cmt_t[:], cmt_d.ap())

        qT = sb.tile([DH, S], f32r, tag="qT")      # q^T: [d, S]
        kT = sb.tile([DH, S], f32r, tag="kT")      # k^T: [d, S]
        vsb = sb.tile([128, NCHUNK * DH], f32r, tag="v")  # v: [S, d] chunked
        recs = sb.tile([128, NCHUNK], f32, tag="recs")    # 1/rowsum per q-chunk

        # ---- Phase A helpers ----
        # XT is consumed in 512-column "pieces". One piece = one 1 MiB DMA
        # bringing all 4 m-tiles into a single [128, 2048] tile (free dim =
        # (m, col)); DMA issue runs one group ahead of the projection
        # matmuls so the PE FIFO never stalls on reads.
        XT_TENSORS = (xtq_d, xtk_d, xtv_d)

        def dma_xt_piece(ti, p):
            t = xt_pool.tile([128, NMT * 512], f32r, tag="xtp")
            src_ap = XT_TENSORS[ti].ap().rearrange(
                "(m p) c -> p m c", p=128)[:, :, 512 * p:512 * (p + 1)]
            dst_ap = t[:].rearrange("p (m c) -> p m c", m=NMT)
            nc.sync.dma_start(dst_ap, src_ap)
            return t

        def mm_qk_piece(ti, p, t):
            w_t, outT = ((wq_t, qT), (wk_t, kT))[ti]
            ps = ps_b.tile([DH, 512], f32, tag="psb")
            for m in range(NMT):
                nc.tensor.matmul(
                    ps[:], w_t[:, m * DH:(m + 1) * DH],
                    t[:, m * 512:(m + 1) * 512],
                    start=(m == 0), stop=(m == NMT - 1),
                )
            _copy(outT[:, 512 * p:512 * (p + 1)], ps[:])

        def mm_v_piece(p, t):
            for cc in range(4):
                c = 4 * p + cc
                ps = ps_a.tile([128, 1024], f32, tag="psa")
                for m in range(NMT):
                    nc.tensor.matmul(
                        ps[:, :DH],
                        t[:, m * 512 + cc * 128:m * 512 + (cc + 1) * 128],
                        wv_t[:, m * DH:(m + 1) * DH],
                        start=(m == 0), stop=(m == NMT - 1),
                    )
                _copy(vsb[:, c * DH:(c + 1) * DH], ps[:, :DH])

        def emit_b1_chunk(i):
            W = 128 * (i + 1)
            nbg = (W + 1023) // 1024
            row = row_pool.tile([128, S], f32, tag="row")
            parts = sm_pool.tile([128, 4], f32, tag="parts")
            for bg in range(nbg):
                w = min(1024, W - bg * 1024)
                ps = ps_a.tile([128, 1024], f32, tag="psa")
                for h0 in range(0, w, 512):
                    hw_ = min(512, w - h0)
                    nc.tensor.matmul(
                        ps[:, h0:h0 + hw_], qT[:, i * 128:(i + 1) * 128],
                        kT[:, bg * 1024 + h0:bg * 1024 + h0 + hw_],
                        start=True, stop=True,
                    )
                if bg == nbg - 1:
                    nc.vector.tensor_add(
                        ps[:, w - 128:w], ps[:, w - 128:w], cm_t[:]
                    )
                nc.scalar.activation(
                    row[:, bg * 1024:bg * 1024 + w], ps[:, :w], Exp,
                    scale=1.0 / DH, accum_out=parts[:, bg:bg + 1],
                )
            rsum = sm_pool.tile([128, 1], f32, tag="rsum")
            nc.vector.reduce_sum(rsum[:], parts[:, :nbg], AX)
            nc.vector.reciprocal(recs[:, i:i + 1], rsum[:])
            for p0 in range(0, W, 2048):
                pw = min(2048, W - p0)
                nc.vector.tensor_scalar_mul(
                    row[:, p0:p0 + pw], row[:, p0:p0 + pw], recs[:, i:i + 1])
                nc.sync.dma_start(
                    scores_d.ap()[i * 128:(i + 1) * 128, p0:p0 + pw],
                    row[:, p0:p0 + pw])

        def emit_st_pair(g, j0):
            """ST matmul pair (k-tiles j0, j0+1) + mask + exp -> est tile."""
            ps = ps_a.tile([128, 1024], f32, tag="psa")
            for u in range(2):
                j = j0 + u
                o = u * 512
                nc.tensor.matmul(
                    ps[:, o:o + 512], kT[:, j * 128:(j + 1) * 128],
                    qT[:, g * 512:(g + 1) * 512],
                    start=True, stop=True,
                )
                dj = j - 4 * g
                if dj >= 0:
                    for cc in range(min(dj, 4)):
                        nc.vector.memset(
                            ps[:, o + cc * 128:o + (cc + 1) * 128], -1e9)
                    if dj <= 3:
                        nc.vector.tensor_add(
                            ps[:, o + dj * 128:o + (dj + 1) * 128],
                            ps[:, o + dj * 128:o + (dj + 1) * 128], cmt_t[:]
                        )
            est = est_pool.tile([128, 1024], f32r, tag="est")
            nc.scalar.activation(est[:], ps[:], Exp, scale=1.0 / DH)
            return est

        def emit_b2_group(g, b1_chunks=None):
            b1_chunks = list(b1_chunks or [])
            ot = ps_b.tile([DH, 512], f32, tag="psb")
            nj = 4 * g + 4
            # software-pipelined: st-pair one step ahead of its av-pair;
            # optionally interleave B1 chunks between pairs
            npairs = nj // 2
            b1_every = max(1, npairs // 4) if b1_chunks else 0
            pend = None  # (j0, est)
            for pi, j0 in enumerate(range(0, nj, 2)):
                if b1_chunks and b1_every and pi % b1_every == 0:
                    emit_b1_chunk(b1_chunks.pop(0)) if b1_chunks else None
                est = emit_st_pair(g, j0)
                if pend is not None:
                    pj, pest = pend
                    for u in range(2):
                        j = pj + u
                        nc.tensor.matmul(
                            ot[:], vsb[:, j * DH:(j + 1) * DH],
                            pest[:, u * 512:(u + 1) * 512],
                            start=(j == 0), stop=False,
                        )
                pend = (j0, est)
            pj, pest = pend
            for u in range(2):
                j = pj + u
                nc.tensor.matmul(
                    ot[:], vsb[:, j * DH:(j + 1) * DH],
                    pest[:, u * 512:(u + 1) * 512],
                    start=(j == 0), stop=(j == nj - 1),
                )
            while b1_chunks:
                emit_b1_chunk(b1_chunks.pop(0))
            ot_sb = ot_sb_pool.tile([DH, 512], f32r, tag="otsb")
            _copy(ot_sb[:], ot[:])
            return ot_sb

        def emit_y(g, ot_sb):
            ysb = ysb_pool.tile([128, 2048], f32, tag="ysb")
            for c4 in range(4):
                i = 4 * g + c4
                ps = ps_a.tile([128, 1024], f32, tag="psa")
                nc.tensor.matmul(
                    ps[:, :512], ot_sb[:, c4 * 128:(c4 + 1) * 128], wo_t[:],
                    start=True, stop=True,
                )
                nc.vector.tensor_scalar_mul(
                    ysb[:, c4 * 512:(c4 + 1) * 512], ps[:, :512],
                    recs[:, i:i + 1])
            dst = y_d.ap()[4 * g * 128:(4 * g + 4) * 128, :].rearrange(
                "(cc p) c -> p cc c", p=128)
            nc.sync.dma_start(dst, ysb[:].rearrange("p (cc c) -> p cc c", cc=4))

        # ---- Emission order: fully incremental, one-group DMA lookahead ----
        tiles = {}
        for p in (0, 1):
            for ti in range(3):
                tiles[(ti, p)] = dma_xt_piece(ti, p)
        pend_y = None
        for g in range(NGRP):
            mm_qk_piece(0, g, tiles.pop((0, g)))
            mm_qk_piece(1, g, tiles.pop((1, g)))
            mm_v_piece(g, tiles.pop((2, g)))
            if g + 2 < NGRP:
                for ti in range(3):
                    tiles[(ti, g + 2)] = dma_xt_piece(ti, g + 2)
            if ORDER.get("b2_split"):
                # interleave B1 chunks with B2 st-pair batches
                ot_sb = emit_b2_group(g, b1_chunks=[4 * g + c for c in range(4)])
            else:
                if ORDER.get("b2_first"):
                    ot_sb = emit_b2_group(g)
                    for c4 in range(4):
                        emit_b1_chunk(4 * g + c4)
                else:
                    for c4 in range(4):
                        emit_b1_chunk(4 * g + c4)
                    ot_sb = emit_b2_group(g)
            if pend_y is not None:
                emit_y(pend_y[0], pend_y[1])
            pend_y = (g, ot_sb)
        emit_y(pend_y[0], pend_y[1])

    nc.compile()
    return nc


_jit_cache = {}


def _run_spmd(nc, in_maps):
    """Run the SPMD kernel on 8 cores.

    Under axon, replicate bass2jax.run_bass_via_pjrt but cache the jitted
    shard_map executable across kernel() calls (run_bass_kernel_spmd
    rebuilds the closure each call, forcing a re-trace). In a native
    environment (real /dev/neuron*), defer to run_bass_kernel_spmd so
    profiling hooks work.
    """
    global LAST_RESULTS
    from concourse._compat import axon_active
    if not axon_active() or TRACE:
        try:
            LAST_RESULTS = run_bass_kernel_spmd(
                nc, in_maps, core_ids=list(range(H)), trace=TRACE,
            )
            return LAST_RESULTS.results
        except ModuleNotFoundError:
            if not axon_active():
                raise
            # axon NTFF hook unavailable -- fall through to untraced path

    import jax
    from jax.sharding import Mesh, PartitionSpec
    try:
        from jax.experimental.shard_map import shard_map
    except ImportError:
        from jax.shard_map import shard_map  # newer jax
    from concourse import bass2jax

    n_cores = len(in_maps)
    key = id(nc)
    if key not in _jit_cache:
        bass2jax.install_neuronx_cc_hook()
        partition_name = (nc.partition_id_tensor.name
                          if nc.partition_id_tensor else None)
        in_names, out_names, out_avals, zero_outs = [], [], [], []
        for alloc in nc.m.functions[0].allocations:
            if not isinstance(alloc, mybir.MemoryLocationSet):
                continue
            name = alloc.memorylocations[0].name
            if alloc.kind == "ExternalInput":
                if name != partition_name:
                    in_names.append(name)
            elif alloc.kind == "ExternalOutput":
                out_names.append(name)
                shape = tuple(alloc.tensor_shape)
                dtype = mybir.dt.np(alloc.dtype)
                out_avals.append(jax.core.ShapedArray(shape, dtype))
                zero_outs.append(np.zeros(shape, dtype))
        n_params = len(in_names)
        all_in = in_names + out_names
        if partition_name is not None:
            all_in = all_in + [partition_name]

        def _body(*args):
            operands = list(args)
            if partition_name is not None:
                operands.append(bass2jax.partition_id_tensor())
            outs = bass2jax._bass_exec_p.bind(
                *operands,
                out_avals=tuple(out_avals),
                in_names=tuple(all_in),
                out_names=tuple(out_names),
                lowering_input_output_aliases=(),
                sim_require_finite=True,
                sim_require_nnan=True,
                nc=nc,
            )
            return tuple(outs)

        devices = jax.devices()[:n_cores]
        mesh = Mesh(np.asarray(devices), ("core",))
        in_specs = (PartitionSpec("core"),) * (n_params + len(out_names))
        out_specs = (PartitionSpec("core"),) * len(out_names)
        donate = tuple(range(n_params, n_params + len(out_names)))
        sharded = jax.jit(
            shard_map(_body, mesh=mesh, in_specs=in_specs,
                      out_specs=out_specs, check_rep=False),
            donate_argnums=donate, keep_unused=True,
        )
        _jit_cache[key] = (sharded, in_names, out_names, out_avals, zero_outs)

    sharded, in_names, out_names, out_avals, zero_outs = _jit_cache[key]
    concat_in = [
        np.concatenate([np.asarray(in_maps[c][n]) for c in range(n_cores)],
                       axis=0)
        for n in in_names
    ]
    concat_zeros = [
        np.zeros((n_cores * z.shape[0], *z.shape[1:]), z.dtype)
        for z in zero_outs
    ]
    out_arrs = sharded(*concat_in, *concat_zeros)
    return [
        {
            name: np.asarray(out_arrs[i]).reshape(
                n_cores, *out_avals[i].shape)[c]
            for i, name in enumerate(out_names)
        }
        for c in range(n_cores)
    ]


def _reference_np(values, keys, queries, mask, Wq, bq, Wk, bk, Wv, bv, Wo, bo):
    """Numpy fallback matching the jax reference (used only if inputs deviate
    from the expected causal-mask/zero-bias setup)."""
    B = queries.shape[0]
    q = (queries @ Wq + bq).reshape(B, S, H, DH).transpose(0, 2, 1, 3)
    k = (keys @ Wk + bk).reshape(B, S, H, DH).transpose(0, 2, 1, 3)
    v = (values @ Wv + bv).reshape(B, S, H, DH).transpose(0, 2, 1, 3)
    sc = np.einsum("bhqd,bhkd->bhqk", q, k) / np.float32(DH)
    sc = sc + mask * np.float32(-1e9)
    sc = sc - sc.max(axis=-1, keepdims=True)
    e = np.exp(sc)
    p = e / e.sum(axis=-1, keepdims=True)
    out = np.einsum("bhqk,bhkd->bhqd", p, v)
    out = out.transpose(0, 2, 1, 3).reshape(B, S, DM)
    return (out @ Wo + bo).astype(np.float32), p.astype(np.float32)


def kernel(values, keys, queries, mask, Wq, bq, Wk, bk, Wv, bv, Wo, bo):
    global LAST_RESULTS
    values = np.asarray(values, dtype=np.float32)
    keys = np.asarray(keys, dtype=np.float32)
    queries = np.asarray(queries, dtype=np.float32)
    mask = np.asarray(mask, dtype=np.float32)
    Wq, bq = np.asarray(Wq, np.float32), np.asarray(bq, np.float32)
    Wk, bk = np.asarray(Wk, np.float32), np.asarray(bk, np.float32)
    Wv, bv = np.asarray(Wv, np.float32), np.asarray(bv, np.float32)
    Wo, bo = np.asarray(Wo, np.float32), np.asarray(bo, np.float32)

    causal = bool(
        queries.shape == (1, S, DM)
        and mask.shape == (1, 1, S, S)
        and not np.any(bq) and not np.any(bk) and not np.any(bv)
        and np.array_equal(mask[0, 0], np.triu(np.ones((S, S), np.float32), k=1))
    )
    if not causal:
        return _reference_np(values, keys, queries, mask,
                             Wq, bq, Wk, bk, Wv, bv, Wo, bo)

    nc = _build()

    xtq = _round_fp32r(queries[0].T)
    xtk = _round_fp32r(keys[0].T)
    xtv = _round_fp32r(values[0].T)
    cm = np.triu(np.full((128, 128), -1e9, np.float32), k=1)
    cmt = np.ascontiguousarray(cm.T)

    def _w_tiles(Wm, h):
        # [DM, DH] slice -> [128, NMT*DH] with col m*DH+d = W[m*128+p, h*DH+d]
        w = Wm[:, h * DH:(h + 1) * DH].reshape(NMT, 128, DH)
        return _round_fp32r(np.ascontiguousarray(w.transpose(1, 0, 2)).reshape(128, NMT * DH))

    in_maps = []
    for h in range(H):
        in_maps.append({
            "xtq": xtq, "xtk": xtk, "xtv": xtv,
            "wq": _w_tiles(Wq, h), "wk": _w_tiles(Wk, h), "wv": _w_tiles(Wv, h),
            "wo": _round_fp32r(np.ascontiguousarray(Wo[h * DH:(h + 1) * DH, :])),
            "cmask": cm, "cmaskt": cmt,
        })

    global _last_in_maps
    _last_in_maps = in_maps
    res = _run_spmd(nc, in_maps)

    scores = np.empty((1, H, S, S), np.float32)
    for h in range(H):
        scores[0, h] = res[h]["scores"]
    out = np.sum(np.stack([res[h]["y"] for h in range(H)]), axis=0,
                 dtype=np.float64)
    out = (out + bo.astype(np.float64)).astype(np.float32)[None]
    return out, scores


# revision 32
# speedup vs baseline: 1.0452x; 1.0452x over previous
"""Multi-head attention (B=1, S=4096, D=512, H=8, causal) on 8 trn2 NeuronCores.

Sharding: one head per core (tensor parallel). Each core:
  - computes its head's q/k/v projections from host-transposed inputs
    (qT/kT in [d, S] layout, v in [S, d] layout -- no on-chip transposes),
  - computes scores = softmax(q k^T / d) in [q, k] layout for the scores
    output (exp via ScalarE with accum_out rowsums; causal blocks skipped,
    unwritten output regions are guaranteed-zero),
  - computes the same scores in [k, q] layout to feed P^T directly into the
    attention-value matmul, then its slice of the Wo projection.
Host gathers: scores stacked over heads; partial outputs summed.

All matmuls run in fp32r (fp32 storage, 11-bit mantissa products, fp32
accumulation) at full PE rate; inputs are pre-rounded on host.
"""
import functools
import numpy as np
from contextlib import ExitStack

import concourse.bass as bass
import concourse.tile as tile
import concourse.mybir as mybir
from concourse import bacc
from concourse.bass_utils import run_bass_kernel_spmd

S = 4096
DM = 512
H = 8
DH = DM // H          # 64
NCHUNK = S // 128     # 32 q-chunks of 128 rows
NGRP = S // 512       # 8 groups of 512
NMT = DM // 128       # 4 m-tiles in the model dim

f32 = mybir.dt.float32
f32r = mybir.dt.float32r
f16 = mybir.dt.float16

TRACE = False          # set by test harness to capture an NTFF profile
LAST_RESULTS = None    # BassKernelResults of the last device run
_last_in_maps = None


def _round_fp32r(x: np.ndarray) -> np.ndarray:
    """Round fp32 -> fp32r (sign + 8e + 11m in the top 20 bits, RNE)."""
    u = np.ascontiguousarray(x, dtype=np.float32).view(np.uint32)
    bias = ((u >> 12) & 1) + np.uint32(0x7FF)
    return (((u + bias) >> 12) << 12).view(np.float32)


ORDER = dict(b2_split=True)

BUILD_CFG = dict(
    xt_bufs=6, row_bufs=4, est_bufs=4, ysb_bufs=2, otsb_bufs=2, sm_bufs=4,
    psa_bufs=3, psb_bufs=2, dve_copies=True,
)


@functools.lru_cache(maxsize=1)
def _build():
    return _build_cfg(**BUILD_CFG)


def _build_cfg(xt_bufs, row_bufs, est_bufs, ysb_bufs, otsb_bufs, sm_bufs,
               psa_bufs, psb_bufs, dve_copies):
    nc = bacc.Bacc("TRN2", target_bir_lowering=False, debug=False, num_devices=8)

    xtq_d = nc.dram_tensor("xtq", [DM, S], f16, kind="ExternalInput")
    xtk_d = nc.dram_tensor("xtk", [DM, S], f16, kind="ExternalInput")
    xtv_d = nc.dram_tensor("xtv", [DM, S], f16, kind="ExternalInput")
    # weight slices pre-arranged on host to [128, NMT*DH]: col m*DH+d = W[m*128+p, d]
    wq_d = nc.dram_tensor("wq", [128, NMT * DH], f16, kind="ExternalInput")
    wk_d = nc.dram_tensor("wk", [128, NMT * DH], f16, kind="ExternalInput")
    wv_d = nc.dram_tensor("wv", [128, NMT * DH], f16, kind="ExternalInput")
    wo_d = nc.dram_tensor("wo", [DH, DM], f32r, kind="ExternalInput")
    cm_d = nc.dram_tensor("cmask", [128, 128], f32, kind="ExternalInput")
    cmt_d = nc.dram_tensor("cmaskt", [128, 128], f32, kind="ExternalInput")

    scores_d = nc.dram_tensor("scores", [S, S], f32, kind="ExternalOutput")
    y_d = nc.dram_tensor("y", [S, DM], f32, kind="ExternalOutput")

    Exp = mybir.ActivationFunctionType.Exp
    AX = mybir.AxisListType.X

    with tile.TileContext(nc) as tc, ExitStack() as ctx:
        sb = ctx.enter_context(tc.tile_pool(name="sb", bufs=1))
        xt_pool = ctx.enter_context(tc.tile_pool(name="xt", bufs=xt_bufs))
        row_pool = ctx.enter_context(tc.tile_pool(name="row", bufs=row_bufs))
        est_pool = ctx.enter_context(tc.tile_pool(name="est", bufs=est_bufs))
        ysb_pool = ctx.enter_context(tc.tile_pool(name="ysb", bufs=ysb_bufs))
        ot_sb_pool = ctx.enter_context(tc.tile_pool(name="otsb", bufs=otsb_bufs))
        sm_pool = ctx.enter_context(tc.tile_pool(name="sm", bufs=sm_bufs))
        ps_a = ctx.enter_context(tc.tile_pool(name="psa", bufs=psa_bufs, space="PSUM"))
        ps_b = ctx.enter_context(tc.tile_pool(name="psb", bufs=psb_bufs, space="PSUM"))
        _copy = nc.vector.tensor_copy if dve_copies else (
            lambda out, in_: nc.scalar.copy(out, in_))

        wq_t = sb.tile([128, NMT * DH], f16, tag="wq")
        wk_t = sb.tile([128, NMT * DH], f16, tag="wk")
        wv_t = sb.tile([128, NMT * DH], f16, tag="wv")
        wo_t = sb.tile([DH, DM], f32r, tag="wo")
        cm_t = sb.tile([128, 128], f32, tag="cm")
        cmt_t = sb.tile([128, 128], f32, tag="cmt")
        nc.sync.dma_start(wq_t[:], wq_d.ap())
        nc.sync.dma_start(wk_t[:], wk_d.ap())
        nc.sync.dma_start(wv_t[:], wv_d.ap())
        nc.sync.dma_start(wo_t[:], wo_d.ap())
        nc.sync.dma_start(cm_t[:], cm_d.ap())
        nc.sync.dma_start(cmt_t[:], cmt_d.ap())

        qT = sb.tile([DH, S], f32r, tag="qT")      # q^T: [d, S]
        kT = sb.tile([DH, S], f32r, tag="kT")      # k^T: [d, S]
        vsb = sb.tile([128, NCHUNK * DH], f32r, tag="v")  # v: [S, d] chunked
        recs = sb.tile([128, NCHUNK], f32, tag="recs")    # 1/rowsum per q-chunk

        # ---- Phase A helpers ----
        # XT is consumed in 512-column "pieces". One piece = one 1 MiB DMA
        # bringing all 4 m-tiles into a single [128, 2048] tile (free dim =
        # (m, col)); DMA issue runs one group ahead of the projection
        # matmuls so the PE FIFO never stalls on reads.
        XT_TENSORS = (xtq_d, xtk_d, xtv_d)

        def dma_xt_piece(ti, p):
            t = xt_pool.tile([128, NMT * 512], f16, tag="xtp")
            src_ap = XT_TENSORS[ti].ap().rearrange(
                "(m p) c -> p m c", p=128)[:, :, 512 * p:512 * (p + 1)]
            dst_ap = t[:].rearrange("p (m c) -> p m c", m=NMT)
            nc.sync.dma_start(dst_ap, src_ap)
            return t

        def mm_qk_piece(ti, p, t):
            w_t, outT = ((wq_t, qT), (wk_t, kT))[ti]
            ps = ps_b.tile([DH, 512], f32, tag="psb")
            for m in range(NMT):
                nc.tensor.matmul(
                    ps[:], w_t[:, m * DH:(m + 1) * DH],
                    t[:, m * 512:(m + 1) * 512],
                    start=(m == 0), stop=(m == NMT - 1),
                )
            _copy(outT[:, 512 * p:512 * (p + 1)], ps[:])

        def mm_v_piece(p, t):
            for cc in range(4):
                c = 4 * p + cc
                ps = ps_a.tile([128, 1024], f32, tag="psa")
                for m in range(NMT):
                    nc.tensor.matmul(
                        ps[:, :DH],
                        t[:, m * 512 + cc * 128:m * 512 + (cc + 1) * 128],
                        wv_t[:, m * DH:(m + 1) * DH],
                        start=(m == 0), stop=(m == NMT - 1),
                    )
                _copy(vsb[:, c * DH:(c + 1) * DH], ps[:, :DH])

        def emit_b1_chunk(i):
            W = 128 * (i + 1)
            nbg = (W + 1023) // 1024
            row = row_pool.tile([128, S], f32, tag="row")
            parts = sm_pool.tile([128, 4], f32, tag="parts")
            for bg in range(nbg):
                w = min(1024, W - bg * 1024)
                ps = ps_a.tile([128, 1024], f32, tag="psa")
                for h0 in range(0, w, 512):
                    hw_ = min(512, w - h0)
                    nc.tensor.matmul(
                        ps[:, h0:h0 + hw_], qT[:, i * 128:(i + 1) * 128],
                        kT[:, bg * 1024 + h0:bg * 1024 + h0 + hw_],
                        start=True, stop=True,
                    )
                if bg == nbg - 1:
                    nc.vector.tensor_add(
                        ps[:, w - 128:w], ps[:, w - 128:w], cm_t[:]
                    )
                nc.scalar.activation(
                    row[:, bg * 1024:bg * 1024 + w], ps[:, :w], Exp,
                    scale=1.0 / DH, accum_out=parts[:, bg:bg + 1],
                )
            rsum = sm_pool.tile([128, 1], f32, tag="rsum")
            nc.vector.reduce_sum(rsum[:], parts[:, :nbg], AX)
            nc.vector.reciprocal(recs[:, i:i + 1], rsum[:])
            for p0 in range(0, W, 2048):
                pw = min(2048, W - p0)
                nc.vector.tensor_scalar_mul(
                    row[:, p0:p0 + pw], row[:, p0:p0 + pw], recs[:, i:i + 1])
                nc.sync.dma_start(
                    scores_d.ap()[i * 128:(i + 1) * 128, p0:p0 + pw],
                    row[:, p0:p0 + pw])

        def emit_st_pair(g, j0):
            """ST matmul pair (k-tiles j0, j0+1) + mask + exp -> est tile."""
            ps = ps_a.tile([128, 1024], f32, tag="psa")
            for u in range(2):
                j = j0 + u
                o = u * 512
                nc.tensor.matmul(
                    ps[:, o:o + 512], kT[:, j * 128:(j + 1) * 128],
                    qT[:, g * 512:(g + 1) * 512],
                    start=True, stop=True,
                )
                dj = j - 4 * g
                if dj >= 0:
                    for cc in range(min(dj, 4)):
                        nc.vector.memset(
                            ps[:, o + cc * 128:o + (cc + 1) * 128], -1e9)
                    if dj <= 3:
                        nc.vector.tensor_add(
                            ps[:, o + dj * 128:o + (dj + 1) * 128],
                            ps[:, o + dj * 128:o + (dj + 1) * 128], cmt_t[:]
                        )
            est = est_pool.tile([128, 1024], f32r, tag="est")
            nc.scalar.activation(est[:], ps[:], Exp, scale=1.0 / DH)
            return est

        def emit_b2_group(g, b1_chunks=None):
            b1_chunks = list(b1_chunks or [])
            ot = ps_b.tile([DH, 512], f32, tag="psb")
            nj = 4 * g + 4
            # software-pipelined: st-pair one step ahead of its av-pair;
            # optionally interleave B1 chunks between pairs
            npairs = nj // 2
            b1_every = max(1, npairs // 4) if b1_chunks else 0
            pend = None  # (j0, est)
            for pi, j0 in enumerate(range(0, nj, 2)):
                if b1_chunks and b1_every and pi % b1_every == 0:
                    emit_b1_chunk(b1_chunks.pop(0)) if b1_chunks else None
                est = emit_st_pair(g, j0)
                if pend is not None:
                    pj, pest = pend
                    for u in range(2):
                        j = pj + u
                        nc.tensor.matmul(
                            ot[:], vsb[:, j * DH:(j + 1) * DH],
                            pest[:, u * 512:(u + 1) * 512],
                            start=(j == 0), stop=False,
                        )
                pend = (j0, est)
            pj, pest = pend
            for u in range(2):
                j = pj + u
                nc.tensor.matmul(
                    ot[:], vsb[:, j * DH:(j + 1) * DH],
                    pest[:, u * 512:(u + 1) * 512],
                    start=(j == 0), stop=(j == nj - 1),
                )
            while b1_chunks:
                emit_b1_chunk(b1_chunks.pop(0))
            ot_sb = ot_sb_pool.tile([DH, 512], f32r, tag="otsb")
            _copy(ot_sb[:], ot[:])
            return ot_sb

        def emit_y(g, ot_sb):
            ysb = ysb_pool.tile([128, 2048], f32, tag="ysb")
            for c4 in range(4):
                i = 4 * g + c4
                ps = ps_a.tile([128, 1024], f32, tag="psa")
                nc.tensor.matmul(
                    ps[:, :512], ot_sb[:, c4 * 128:(c4 + 1) * 128], wo_t[:],
                    start=True, stop=True,
                )
                nc.vector.tensor_scalar_mul(
                    ysb[:, c4 * 512:(c4 + 1) * 512], ps[:, :512],
                    recs[:, i:i + 1])
            dst = y_d.ap()[4 * g * 128:(4 * g + 4) * 128, :].rearrange(
                "(cc p) c -> p cc c", p=128)
            nc.sync.dma_start(dst, ysb[:].rearrange("p (cc c) -> p cc c", cc=4))

        # ---- Emission order: fully incremental, one-group DMA lookahead ----
        LA = ORDER.get("lookahead", 2)
        tiles = {}
        for p in range(min(LA, NGRP)):
            for ti in range(3):
                tiles[(ti, p)] = dma_xt_piece(ti, p)
        pend_y = None
        for g in range(NGRP):
            mm_qk_piece(0, g, tiles.pop((0, g)))
            mm_qk_piece(1, g, tiles.pop((1, g)))
            mm_v_piece(g, tiles.pop((2, g)))
            if g + LA < NGRP:
                for ti in range(3):
                    tiles[(ti, g + LA)] = dma_xt_piece(ti, g + LA)
            if ORDER.get("b2_split"):
                # interleave B1 chunks with B2 st-pair batches
                ot_sb = emit_b2_group(g, b1_chunks=[4 * g + c for c in range(4)])
            else:
                if ORDER.get("b2_first"):
                    ot_sb = emit_b2_group(g)
                    for c4 in range(4):
                        emit_b1_chunk(4 * g + c4)
                else:
                    for c4 in range(4):
                        emit_b1_chunk(4 * g + c4)
                    ot_sb = emit_b2_group(g)
            if pend_y is not None:
                emit_y(pend_y[0], pend_y[1])
            pend_y = (g, ot_sb)
        emit_y(pend_y[0], pend_y[1])

    nc.compile()
    return nc


_jit_cache = {}


def _run_spmd(nc, in_maps):
    """Run the SPMD kernel on 8 cores.

    Under axon, replicate bass2jax.run_bass_via_pjrt but cache the jitted
    shard_map executable across kernel() calls (run_bass_kernel_spmd
    rebuilds the closure each call, forcing a re-trace). In a native
    environment (real /dev/neuron*), defer to run_bass_kernel_spmd so
    profiling hooks work.
    """
    global LAST_RESULTS
    from concourse._compat import axon_active
    if not axon_active() or TRACE:
        try:
            LAST_RESULTS = run_bass_kernel_spmd(
                nc, in_maps, core_ids=list(range(H)), trace=TRACE,
            )
            return LAST_RESULTS.results
        except ModuleNotFoundError:
            if not axon_active():
                raise
            # axon NTFF hook unavailable -- fall through to untraced path

    import jax
    from jax.sharding import Mesh, PartitionSpec
    try:
        from jax.experimental.shard_map import shard_map
    except ImportError:
        from jax.shard_map import shard_map  # newer jax
    from concourse import bass2jax

    n_cores = len(in_maps)
    key = id(nc)
    if key not in _jit_cache:
        bass2jax.install_neuronx_cc_hook()
        partition_name = (nc.partition_id_tensor.name
                          if nc.partition_id_tensor else None)
        in_names, out_names, out_avals, zero_outs = [], [], [], []
        for alloc in nc.m.functions[0].allocations:
            if not isinstance(alloc, mybir.MemoryLocationSet):
                continue
            name = alloc.memorylocations[0].name
            if alloc.kind == "ExternalInput":
                if name != partition_name:
                    in_names.append(name)
            elif alloc.kind == "ExternalOutput":
                out_names.append(name)
                shape = tuple(alloc.tensor_shape)
                dtype = mybir.dt.np(alloc.dtype)
                out_avals.append(jax.core.ShapedArray(shape, dtype))
                zero_outs.append(np.zeros(shape, dtype))
        n_params = len(in_names)
        all_in = in_names + out_names
        if partition_name is not None:
            all_in = all_in + [partition_name]

        def _body(*args):
            operands = list(args)
            if partition_name is not None:
                operands.append(bass2jax.partition_id_tensor())
            outs = bass2jax._bass_exec_p.bind(
                *operands,
                out_avals=tuple(out_avals),
                in_names=tuple(all_in),
                out_names=tuple(out_names),
                lowering_input_output_aliases=(),
                sim_require_finite=True,
                sim_require_nnan=True,
                nc=nc,
            )
            return tuple(outs)

        devices = jax.devices()[:n_cores]
        mesh = Mesh(np.asarray(devices), ("core",))
        in_specs = (PartitionSpec("core"),) * (n_params + len(out_names))
        out_specs = (PartitionSpec("core"),) * len(out_names)
        donate = tuple(range(n_params, n_params + len(out_names)))
        sharded = jax.jit(
            shard_map(_body, mesh=mesh, in_specs=in_specs,
                      out_specs=out_specs, check_rep=False),
            donate_argnums=donate, keep_unused=True,
        )
        _jit_cache[key] = (sharded, in_names, out_names, out_avals, zero_outs)

    sharded, in_names, out_names, out_avals, zero_outs = _jit_cache[key]
    concat_in = [
        np.concatenate([np.asarray(in_maps[c][n]) for c in range(n_cores)],
                       axis=0)
        for n in in_names
    ]
    concat_zeros = [
        np.zeros((n_cores * z.shape[0], *z.shape[1:]), z.dtype)
        for z in zero_outs
    ]
    out_arrs = sharded(*concat_in, *concat_zeros)
    return [
        {
            name: np.asarray(out_arrs[i]).reshape(
                n_cores, *out_avals[i].shape)[c]
            for i, name in enumerate(out_names)
        }
        for c in range(n_cores)
    ]


def _reference_np(values, keys, queries, mask, Wq, bq, Wk, bk, Wv, bv, Wo, bo):
    """Numpy fallback matching the jax reference (used only if inputs deviate
    from the expected causal-mask/zero-bias setup)."""
    B = queries.shape[0]
    q = (queries @ Wq + bq).reshape(B, S, H, DH).transpose(0, 2, 1, 3)
    k = (keys @ Wk + bk).reshape(B, S, H, DH).transpose(0, 2, 1, 3)
    v = (values @ Wv + bv).reshape(B, S, H, DH).transpose(0, 2, 1, 3)
    sc = np.einsum("bhqd,bhkd->bhqk", q, k) / np.float32(DH)
    sc = sc + mask * np.float32(-1e9)
    sc = sc - sc.max(axis=-1, keepdims=True)
    e = np.exp(sc)
    p = e / e.sum(axis=-1, keepdims=True)
    out = np.einsum("bhqk,bhkd->bhqd", p, v)
    out = out.transpose(0, 2, 1, 3).reshape(B, S, DM)
    return (out @ Wo + bo).astype(np.float32), p.astype(np.float32)


def kernel(values, keys, queries, mask, Wq, bq, Wk, bk, Wv, bv, Wo, bo):
    global LAST_RESULTS
    values = np.asarray(values, dtype=np.float32)
    keys = np.asarray(keys, dtype=np.float32)
    queries = np.asarray(queries, dtype=np.float32)
    mask = np.asarray(mask, dtype=np.float32)
    Wq, bq = np.asarray(Wq, np.float32), np.asarray(bq, np.float32)
    Wk, bk = np.asarray(Wk, np.float32), np.asarray(bk, np.float32)
    Wv, bv = np.asarray(Wv, np.float32), np.asarray(bv, np.float32)
    Wo, bo = np.asarray(Wo, np.float32), np.asarray(bo, np.float32)

    causal = bool(
        queries.shape == (1, S, DM)
        and mask.shape == (1, 1, S, S)
        and not np.any(bq) and not np.any(bk) and not np.any(bv)
        and np.array_equal(mask[0, 0], np.triu(np.ones((S, S), np.float32), k=1))
    )
    if not causal:
        return _reference_np(values, keys, queries, mask,
                             Wq, bq, Wk, bk, Wv, bv, Wo, bo)

    nc = _build()

    xtq = np.ascontiguousarray(queries[0].T).astype(np.float16)
    xtk = np.ascontiguousarray(keys[0].T).astype(np.float16)
    xtv = np.ascontiguousarray(values[0].T).astype(np.float16)
    cm = np.triu(np.full((128, 128), -1e9, np.float32), k=1)
    cmt = np.ascontiguousarray(cm.T)

    def _w_tiles(Wm, h):
        # [DM, DH] slice -> [128, NMT*DH] with col m*DH+d = W[m*128+p, h*DH+d]
        w = Wm[:, h * DH:(h + 1) * DH].reshape(NMT, 128, DH)
        return np.ascontiguousarray(
            w.transpose(1, 0, 2)).reshape(128, NMT * DH).astype(np.float16)

    in_maps = []
    for h in range(H):
        in_maps.append({
            "xtq": xtq, "xtk": xtk, "xtv": xtv,
            "wq": _w_tiles(Wq, h), "wk": _w_tiles(Wk, h), "wv": _w_tiles(Wv, h),
            "wo": _round_fp32r(np.ascontiguousarray(Wo[h * DH:(h + 1) * DH, :])),
            "cmask": cm, "cmaskt": cmt,
        })

    global _last_in_maps
    _last_in_maps = in_maps
    res = _run_spmd(nc, in_maps)

    scores = np.empty((1, H, S, S), np.float32)
    for h in range(H):
        scores[0, h] = res[h]["scores"]
    out = np.sum(np.stack([res[h]["y"] for h in range(H)]), axis=0,
                 dtype=np.float64)
    out = (out + bo.astype(np.float64)).astype(np.float32)[None]
    return out, scores


# revision 34
# speedup vs baseline: 1.1009x; 1.0533x over previous
"""Multi-head attention (B=1, S=4096, D=512, H=8, causal) on 8 trn2 NeuronCores.

Sharding: one head per core (tensor parallel). Each core:
  - computes its head's q/k/v projections from host-transposed inputs
    (qT/kT in [d, S] layout, v in [S, d] layout -- no on-chip transposes),
  - computes scores = softmax(q k^T / d) in [q, k] layout for the scores
    output (exp via ScalarE with accum_out rowsums; causal blocks skipped,
    unwritten output regions are guaranteed-zero),
  - computes the same scores in [k, q] layout to feed P^T directly into the
    attention-value matmul, then its slice of the Wo projection.
Host gathers: scores stacked over heads; partial outputs summed.

All matmuls run in fp32r (fp32 storage, 11-bit mantissa products, fp32
accumulation) at full PE rate; inputs are pre-rounded on host.
"""
import functools
import numpy as np
from contextlib import ExitStack

import concourse.bass as bass
import concourse.tile as tile
import concourse.mybir as mybir
from concourse import bacc
from concourse.bass_utils import run_bass_kernel_spmd

S = 4096
DM = 512
H = 8
DH = DM // H          # 64
NCHUNK = S // 128     # 32 q-chunks of 128 rows
NGRP = S // 512       # 8 groups of 512
NMT = DM // 128       # 4 m-tiles in the model dim

f32 = mybir.dt.float32
f32r = mybir.dt.float32r
f16 = mybir.dt.float16

TRACE = False          # set by test harness to capture an NTFF profile
LAST_RESULTS = None    # BassKernelResults of the last device run
_last_in_maps = None


def _round_fp32r(x: np.ndarray) -> np.ndarray:
    """Round fp32 -> fp32r (sign + 8e + 11m in the top 20 bits, RNE)."""
    u = np.ascontiguousarray(x, dtype=np.float32).view(np.uint32)
    bias = ((u >> 12) & 1) + np.uint32(0x7FF)
    return (((u + bias) >> 12) << 12).view(np.float32)


ORDER = dict(b2_split=True)

BUILD_CFG = dict(
    xt_bufs=6, row_bufs=4, est_bufs=4, ysb_bufs=2, otsb_bufs=2, sm_bufs=4,
    psa_bufs=3, psb_bufs=2, dve_copies=True,
)


@functools.lru_cache(maxsize=1)
def _build():
    return _build_cfg(**BUILD_CFG)


def _build_cfg(xt_bufs, row_bufs, est_bufs, ysb_bufs, otsb_bufs, sm_bufs,
               psa_bufs, psb_bufs, dve_copies):
    nc = bacc.Bacc("TRN2", target_bir_lowering=False, debug=False, num_devices=8)

    xtq_d = nc.dram_tensor("xtq", [DM, S], f16, kind="ExternalInput")
    xtk_d = nc.dram_tensor("xtk", [DM, S], f16, kind="ExternalInput")
    xtv_d = nc.dram_tensor("xtv", [DM, S], f16, kind="ExternalInput")
    # weight slices pre-arranged on host to [128, NMT*DH]: col m*DH+d = W[m*128+p, d]
    wq_d = nc.dram_tensor("wq", [128, NMT * DH], f16, kind="ExternalInput")
    wk_d = nc.dram_tensor("wk", [128, NMT * DH], f16, kind="ExternalInput")
    wv_d = nc.dram_tensor("wv", [128, NMT * DH], f16, kind="ExternalInput")
    wo_d = nc.dram_tensor("wo", [DH, DM], f32r, kind="ExternalInput")
    cm_d = nc.dram_tensor("cmask", [128, 128], f32r, kind="ExternalInput")
    cmrow_d = nc.dram_tensor("cmrow", [128, 512], f32r, kind="ExternalInput")
    id_d = nc.dram_tensor("ident", [128, 128], f32r, kind="ExternalInput")

    scores_d = nc.dram_tensor("scores", [S, S], f32, kind="ExternalOutput")
    y_d = nc.dram_tensor("y", [S, DM], f32, kind="ExternalOutput")

    Exp = mybir.ActivationFunctionType.Exp
    AX = mybir.AxisListType.X

    with tile.TileContext(nc) as tc, ExitStack() as ctx:
        sb = ctx.enter_context(tc.tile_pool(name="sb", bufs=1))
        xt_pool = ctx.enter_context(tc.tile_pool(name="xt", bufs=xt_bufs))
        row_pool = ctx.enter_context(tc.tile_pool(name="row", bufs=row_bufs))
        est_pool = ctx.enter_context(tc.tile_pool(name="est", bufs=est_bufs))
        ysb_pool = ctx.enter_context(tc.tile_pool(name="ysb", bufs=ysb_bufs))
        ot_sb_pool = ctx.enter_context(tc.tile_pool(name="otsb", bufs=otsb_bufs))
        sm_pool = ctx.enter_context(tc.tile_pool(name="sm", bufs=sm_bufs))
        ps_a = ctx.enter_context(tc.tile_pool(name="psa", bufs=psa_bufs, space="PSUM"))
        ps_b = ctx.enter_context(tc.tile_pool(name="psb", bufs=psb_bufs, space="PSUM"))
        _copy = nc.vector.tensor_copy if dve_copies else (
            lambda out, in_: nc.scalar.copy(out, in_))

        wq_t = sb.tile([128, NMT * DH], f16, tag="wq")
        wk_t = sb.tile([128, NMT * DH], f16, tag="wk")
        wv_t = sb.tile([128, NMT * DH], f16, tag="wv")
        wo_t = sb.tile([DH, DM], f32r, tag="wo")
        cm_t = sb.tile([128, 128], f32r, tag="cm")
        cmrow_t = sb.tile([128, 512], f32r, tag="cmrow")
        id_t = sb.tile([128, 128], f32r, tag="ident")
        nc.sync.dma_start(wq_t[:], wq_d.ap())
        nc.sync.dma_start(wk_t[:], wk_d.ap())
        nc.sync.dma_start(wv_t[:], wv_d.ap())
        nc.sync.dma_start(wo_t[:], wo_d.ap())
        nc.sync.dma_start(cm_t[:], cm_d.ap())
        nc.sync.dma_start(cmrow_t[:], cmrow_d.ap())
        nc.sync.dma_start(id_t[:], id_d.ap())

        qT = sb.tile([DH, S], f32r, tag="qT")      # q^T: [d, S]
        kT = sb.tile([DH, S], f32r, tag="kT")      # k^T: [d, S]
        vsb = sb.tile([128, NCHUNK * DH], f32r, tag="v")  # v: [S, d] chunked
        recs = sb.tile([128, NCHUNK], f32, tag="recs")    # 1/rowsum per q-chunk

        # ---- Phase A helpers ----
        # XT is consumed in 512-column "pieces". One piece = one 1 MiB DMA
        # bringing all 4 m-tiles into a single [128, 2048] tile (free dim =
        # (m, col)); DMA issue runs one group ahead of the projection
        # matmuls so the PE FIFO never stalls on reads.
        XT_TENSORS = (xtq_d, xtk_d, xtv_d)

        def dma_xt_piece(ti, p):
            t = xt_pool.tile([128, NMT * 512], f16, tag="xtp")
            src_ap = XT_TENSORS[ti].ap().rearrange(
                "(m p) c -> p m c", p=128)[:, :, 512 * p:512 * (p + 1)]
            dst_ap = t[:].rearrange("p (m c) -> p m c", m=NMT)
            nc.sync.dma_start(dst_ap, src_ap)
            return t

        def mm_qk_piece(ti, p, t):
            w_t, outT = ((wq_t, qT), (wk_t, kT))[ti]
            ps = ps_b.tile([DH, 512], f32, tag="psb")
            for m in range(NMT):
                nc.tensor.matmul(
                    ps[:], w_t[:, m * DH:(m + 1) * DH],
                    t[:, m * 512:(m + 1) * 512],
                    start=(m == 0), stop=(m == NMT - 1),
                )
            _copy(outT[:, 512 * p:512 * (p + 1)], ps[:])

        def mm_v_piece(p, t):
            for cc in range(4):
                c = 4 * p + cc
                ps = ps_a.tile([128, 1024], f32, tag="psa")
                for m in range(NMT):
                    nc.tensor.matmul(
                        ps[:, :DH],
                        t[:, m * 512 + cc * 128:m * 512 + (cc + 1) * 128],
                        wv_t[:, m * DH:(m + 1) * DH],
                        start=(m == 0), stop=(m == NMT - 1),
                    )
                _copy(vsb[:, c * DH:(c + 1) * DH], ps[:, :DH])

        def emit_b1_chunk(i):
            W = 128 * (i + 1)
            nbg = (W + 1023) // 1024
            row = row_pool.tile([128, S], f32, tag="row")
            parts = sm_pool.tile([128, 4], f32, tag="parts")
            for bg in range(nbg):
                w = min(1024, W - bg * 1024)
                ps = ps_a.tile([128, 1024], f32, tag="psa")
                for h0 in range(0, w, 512):
                    hw_ = min(512, w - h0)
                    is_diag = (bg == nbg - 1) and (h0 + hw_ == w)
                    nc.tensor.matmul(
                        ps[:, h0:h0 + hw_], qT[:, i * 128:(i + 1) * 128],
                        kT[:, bg * 1024 + h0:bg * 1024 + h0 + hw_],
                        start=True, stop=not is_diag,
                    )
                    if is_diag:
                        # accumulate the additive causal mask on the PE so
                        # the exp never waits on a DVE hop
                        nc.tensor.matmul(
                            ps[:, w - 128:w], id_t[:], cm_t[:],
                            start=False, stop=True,
                        )
                nc.scalar.activation(
                    row[:, bg * 1024:bg * 1024 + w], ps[:, :w], Exp,
                    scale=1.0 / DH, accum_out=parts[:, bg:bg + 1],
                )
            rsum = sm_pool.tile([128, 1], f32, tag="rsum")
            nc.vector.reduce_sum(rsum[:], parts[:, :nbg], AX)
            nc.vector.reciprocal(recs[:, i:i + 1], rsum[:])
            for p0 in range(0, W, 2048):
                pw = min(2048, W - p0)
                nc.vector.tensor_scalar_mul(
                    row[:, p0:p0 + pw], row[:, p0:p0 + pw], recs[:, i:i + 1])
                nc.sync.dma_start(
                    scores_d.ap()[i * 128:(i + 1) * 128, p0:p0 + pw],
                    row[:, p0:p0 + pw])

        def emit_st_pair(g, j0):
            """ST matmul pair (k-tiles j0, j0+1) + mask + exp -> est tile."""
            ps = ps_a.tile([128, 1024], f32, tag="psa")
            for u in range(2):
                j = j0 + u
                o = u * 512
                dj = j - 4 * g
                has_mask = 0 <= dj <= 3
                nc.tensor.matmul(
                    ps[:, o:o + 512], kT[:, j * 128:(j + 1) * 128],
                    qT[:, g * 512:(g + 1) * 512],
                    start=True, stop=not has_mask,
                )
                if has_mask:
                    # columns [0, (dj+1)*128): subtiles below the diagonal get
                    # -1e9 everywhere, the diagonal subtile gets the causal
                    # additive mask -- one packed PE accumulate from cmrow
                    wdt = (dj + 1) * 128
                    nc.tensor.matmul(
                        ps[:, o:o + wdt], id_t[:],
                        cmrow_t[:, 512 - wdt:512],
                        start=False, stop=True,
                    )
            est = est_pool.tile([128, 1024], f32r, tag="est")
            nc.scalar.activation(est[:], ps[:], Exp, scale=1.0 / DH)
            return est

        def emit_b2_group(g, b1_chunks=None):
            b1_chunks = list(b1_chunks or [])
            ot = ps_b.tile([DH, 512], f32, tag="psb")
            nj = 4 * g + 4
            # software-pipelined: st-pair one step ahead of its av-pair;
            # optionally interleave B1 chunks between pairs
            npairs = nj // 2
            b1_every = max(1, npairs // 4) if b1_chunks else 0
            pend = None  # (j0, est)
            for pi, j0 in enumerate(range(0, nj, 2)):
                if b1_chunks and b1_every and pi % b1_every == 0:
                    emit_b1_chunk(b1_chunks.pop(0)) if b1_chunks else None
                est = emit_st_pair(g, j0)
                if pend is not None:
                    pj, pest = pend
                    for u in range(2):
                        j = pj + u
                        nc.tensor.matmul(
                            ot[:], vsb[:, j * DH:(j + 1) * DH],
                            pest[:, u * 512:(u + 1) * 512],
                            start=(j == 0), stop=False,
                        )
                pend = (j0, est)
            pj, pest = pend
            for u in range(2):
                j = pj + u
                nc.tensor.matmul(
                    ot[:], vsb[:, j * DH:(j + 1) * DH],
                    pest[:, u * 512:(u + 1) * 512],
                    start=(j == 0), stop=(j == nj - 1),
                )
            while b1_chunks:
                emit_b1_chunk(b1_chunks.pop(0))
            ot_sb = ot_sb_pool.tile([DH, 512], f32r, tag="otsb")
            _copy(ot_sb[:], ot[:])
            return ot_sb

        def emit_y(g, ot_sb):
            ysb = ysb_pool.tile([128, 2048], f32, tag="ysb")
            for c4 in range(4):
                i = 4 * g + c4
                ps = ps_a.tile([128, 1024], f32, tag="psa")
                nc.tensor.matmul(
                    ps[:, :512], ot_sb[:, c4 * 128:(c4 + 1) * 128], wo_t[:],
                    start=True, stop=True,
                )
                nc.vector.tensor_scalar_mul(
                    ysb[:, c4 * 512:(c4 + 1) * 512], ps[:, :512],
                    recs[:, i:i + 1])
            dst = y_d.ap()[4 * g * 128:(4 * g + 4) * 128, :].rearrange(
                "(cc p) c -> p cc c", p=128)
            nc.sync.dma_start(dst, ysb[:].rearrange("p (cc c) -> p cc c", cc=4))

        # ---- Emission order: fully incremental, one-group DMA lookahead ----
        LA = ORDER.get("lookahead", 2)
        tiles = {}
        for p in range(min(LA, NGRP)):
            for ti in range(3):
                tiles[(ti, p)] = dma_xt_piece(ti, p)
        pend_y = None
        for g in range(NGRP):
            mm_qk_piece(0, g, tiles.pop((0, g)))
            mm_qk_piece(1, g, tiles.pop((1, g)))
            mm_v_piece(g, tiles.pop((2, g)))
            if g + LA < NGRP:
                for ti in range(3):
                    tiles[(ti, g + LA)] = dma_xt_piece(ti, g + LA)
            if ORDER.get("b2_split"):
                # interleave B1 chunks with B2 st-pair batches
                ot_sb = emit_b2_group(g, b1_chunks=[4 * g + c for c in range(4)])
            else:
                if ORDER.get("b2_first"):
                    ot_sb = emit_b2_group(g)
                    for c4 in range(4):
                        emit_b1_chunk(4 * g + c4)
                else:
                    for c4 in range(4):
                        emit_b1_chunk(4 * g + c4)
                    ot_sb = emit_b2_group(g)
            if pend_y is not None:
                emit_y(pend_y[0], pend_y[1])
            pend_y = (g, ot_sb)
        emit_y(pend_y[0], pend_y[1])

    nc.compile()
    return nc


_jit_cache = {}


def _run_spmd(nc, in_maps):
    """Run the SPMD kernel on 8 cores.

    Under axon, replicate bass2jax.run_bass_via_pjrt but cache the jitted
    shard_map executable across kernel() calls (run_bass_kernel_spmd
    rebuilds the closure each call, forcing a re-trace). In a native
    environment (real /dev/neuron*), defer to run_bass_kernel_spmd so
    profiling hooks work.
    """
    global LAST_RESULTS
    from concourse._compat import axon_active
    if not axon_active() or TRACE:
        try:
            LAST_RESULTS = run_bass_kernel_spmd(
                nc, in_maps, core_ids=list(range(H)), trace=TRACE,
            )
            return LAST_RESULTS.results
        except ModuleNotFoundError:
            if not axon_active():
                raise
            # axon NTFF hook unavailable -- fall through to untraced path

    import jax
    from jax.sharding import Mesh, PartitionSpec
    try:
        from jax.experimental.shard_map import shard_map
    except ImportError:
        from jax.shard_map import shard_map  # newer jax
    from concourse import bass2jax

    n_cores = len(in_maps)
    key = id(nc)
    if key not in _jit_cache:
        bass2jax.install_neuronx_cc_hook()
        partition_name = (nc.partition_id_tensor.name
                          if nc.partition_id_tensor else None)
        in_names, out_names, out_avals, zero_outs = [], [], [], []
        for alloc in nc.m.functions[0].allocations:
            if not isinstance(alloc, mybir.MemoryLocationSet):
                continue
            name = alloc.memorylocations[0].name
            if alloc.kind == "ExternalInput":
                if name != partition_name:
                    in_names.append(name)
            elif alloc.kind == "ExternalOutput":
                out_names.append(name)
                shape = tuple(alloc.tensor_shape)
                dtype = mybir.dt.np(alloc.dtype)
                out_avals.append(jax.core.ShapedArray(shape, dtype))
                zero_outs.append(np.zeros(shape, dtype))
        n_params = len(in_names)
        all_in = in_names + out_names
        if partition_name is not None:
            all_in = all_in + [partition_name]

        def _body(*args):
            operands = list(args)
            if partition_name is not None:
                operands.append(bass2jax.partition_id_tensor())
            outs = bass2jax._bass_exec_p.bind(
                *operands,
                out_avals=tuple(out_avals),
                in_names=tuple(all_in),
                out_names=tuple(out_names),
                lowering_input_output_aliases=(),
                sim_require_finite=True,
                sim_require_nnan=True,
                nc=nc,
            )
            return tuple(outs)

        devices = jax.devices()[:n_cores]
        mesh = Mesh(np.asarray(devices), ("core",))
        in_specs = (PartitionSpec("core"),) * (n_params + len(out_names))
        out_specs = (PartitionSpec("core"),) * len(out_names)
        donate = tuple(range(n_params, n_params + len(out_names)))
        sharded = jax.jit(
            shard_map(_body, mesh=mesh, in_specs=in_specs,
                      out_specs=out_specs, check_rep=False),
            donate_argnums=donate, keep_unused=True,
        )
        _jit_cache[key] = (sharded, in_names, out_names, out_avals, zero_outs)

    sharded, in_names, out_names, out_avals, zero_outs = _jit_cache[key]
    concat_in = [
        np.concatenate([np.asarray(in_maps[c][n]) for c in range(n_cores)],
                       axis=0)
        for n in in_names
    ]
    concat_zeros = [
        np.zeros((n_cores * z.shape[0], *z.shape[1:]), z.dtype)
        for z in zero_outs
    ]
    out_arrs = sharded(*concat_in, *concat_zeros)
    return [
        {
            name: np.asarray(out_arrs[i]).reshape(
                n_cores, *out_avals[i].shape)[c]
            for i, name in enumerate(out_names)
        }
        for c in range(n_cores)
    ]


def _reference_np(values, keys, queries, mask, Wq, bq, Wk, bk, Wv, bv, Wo, bo):
    """Numpy fallback matching the jax reference (used only if inputs deviate
    from the expected causal-mask/zero-bias setup)."""
    B = queries.shape[0]
    q = (queries @ Wq + bq).reshape(B, S, H, DH).transpose(0, 2, 1, 3)
    k = (keys @ Wk + bk).reshape(B, S, H, DH).transpose(0, 2, 1, 3)
    v = (values @ Wv + bv).reshape(B, S, H, DH).transpose(0, 2, 1, 3)
    sc = np.einsum("bhqd,bhkd->bhqk", q, k) / np.float32(DH)
    sc = sc + mask * np.float32(-1e9)
    sc = sc - sc.max(axis=-1, keepdims=True)
    e = np.exp(sc)
    p = e / e.sum(axis=-1, keepdims=True)
    out = np.einsum("bhqk,bhkd->bhqd", p, v)
    out = out.transpose(0, 2, 1, 3).reshape(B, S, DM)
    return (out @ Wo + bo).astype(np.float32), p.astype(np.float32)


def kernel(values, keys, queries, mask, Wq, bq, Wk, bk, Wv, bv, Wo, bo):
    global LAST_RESULTS
    values = np.asarray(values, dtype=np.float32)
    keys = np.asarray(keys, dtype=np.float32)
    queries = np.asarray(queries, dtype=np.float32)
    mask = np.asarray(mask, dtype=np.float32)
    Wq, bq = np.asarray(Wq, np.float32), np.asarray(bq, np.float32)
    Wk, bk = np.asarray(Wk, np.float32), np.asarray(bk, np.float32)
    Wv, bv = np.asarray(Wv, np.float32), np.asarray(bv, np.float32)
    Wo, bo = np.asarray(Wo, np.float32), np.asarray(bo, np.float32)

    causal = bool(
        queries.shape == (1, S, DM)
        and mask.shape == (1, 1, S, S)
        and not np.any(bq) and not np.any(bk) and not np.any(bv)
        and np.array_equal(mask[0, 0], np.triu(np.ones((S, S), np.float32), k=1))
    )
    if not causal:
        return _reference_np(values, keys, queries, mask,
                             Wq, bq, Wk, bk, Wv, bv, Wo, bo)

    nc = _build()

    xtq = np.ascontiguousarray(queries[0].T).astype(np.float16)
    xtk = np.ascontiguousarray(keys[0].T).astype(np.float16)
    xtv = np.ascontiguousarray(values[0].T).astype(np.float16)
    cm = _round_fp32r(np.triu(np.full((128, 128), -1e9, np.float32), k=1))
    cmt = np.ascontiguousarray(cm.T)
    cmrow = np.concatenate(
        [np.full((128, 384), -1e9, np.float32), cmt], axis=1)
    cmrow = _round_fp32r(np.ascontiguousarray(cmrow))
    ident = np.eye(128, dtype=np.float32)

    def _w_tiles(Wm, h):
        # [DM, DH] slice -> [128, NMT*DH] with col m*DH+d = W[m*128+p, h*DH+d]
        w = Wm[:, h * DH:(h + 1) * DH].reshape(NMT, 128, DH)
        return np.ascontiguousarray(
            w.transpose(1, 0, 2)).reshape(128, NMT * DH).astype(np.float16)

    in_maps = []
    for h in range(H):
        in_maps.append({
            "xtq": xtq, "xtk": xtk, "xtv": xtv,
            "wq": _w_tiles(Wq, h), "wk": _w_tiles(Wk, h), "wv": _w_tiles(Wv, h),
            "wo": _round_fp32r(np.ascontiguousarray(Wo[h * DH:(h + 1) * DH, :])),
            "cmask": cm, "cmrow": cmrow, "ident": ident,
        })

    global _last_in_maps
    _last_in_maps = in_maps
    res = _run_spmd(nc, in_maps)

    scores = np.empty((1, H, S, S), np.float32)
    for h in range(H):
        scores[0, h] = res[h]["scores"]
    out = np.sum(np.stack([res[h]["y"] for h in range(H)]), axis=0,
                 dtype=np.float64)
    out = (out + bo.astype(np.float64)).astype(np.float32)[None]
    return out, scores
